# revision 12
# baseline (speedup 1.0000x reference)
"""Trainium2 Bass kernel for the planar normalizing-flow batch evaluation.

Math (per batch element z in R^2, WIDTH=64 planar units with params u, w, b
derived from a tiny hypernetwork on scalar t):
    lin_k  = w_k . z + b_k
    th_k   = tanh(lin_k)
    dz     = (1/64) sum_k th_k * u_k                  -> [B, 2]
    dlogp  = (sum_k wu_k*th_k^2 - sum_k wu_k) / 64    -> [B, 1]   (wu_k = w_k . u_k)

Distribution: pure data parallel over 8 NeuronCores, batch split evenly.
The hypernetwork (few-microsecond scalar work) runs on host in float32.

Device layout per core (batch slice of 125k elems, split into halves A/B of
62500, processed as 124 column-chunks of 512 pairs, one pair = (A elem, B elem)):
  MM1  : lin[128, n] = lhs1[4,128]^T @ zp[4, n]   (rows 0-63 = A-half lin,
         rows 64-127 = B-half lin; K=4 packs both halves' (z0, z1))
  ACT  : th = tanh(lin + b) (f32r out)
  VE   : sq = th * th
  MM2  : po[0:6, 512c:512c+512] = u / wu contractions of th / sq
         (K=128, M=6, PSUM accumulation; bank c per chunk of the group)
  ACT/VE: F = po * (1/64) + bias_out  (Identity / tensor_scalar, alternating)
  DMA  : F [6, 2048] -> out[og]
All matmuls use float32r (TF32-like, ~13 mantissa bits) at 1 PE cycle/column.
"""
import sys

sys.path.insert(0, "/opt/trn_rl_repo")

import numpy as np

B = 1_000_000
D = 2
WIDTH = 64
NCORES = 8
BC = B // NCORES            # 125000 per core
PH = BC // 2                # 62500 pairs per core
CHUNK = 512
NCHUNK = 124                # ceil(62500/512) padded to a multiple of 4
PPAD = NCHUNK * CHUNK       # 63488
NOG = NCHUNK // 4           # 31 output groups (4 chunks share one PSUM out bank)
NC_CONST = 144              # consts tile columns

_built = {}


def _build_module(repeat=1):
    import concourse.bacc as bacc
    import concourse.tile as tile
    from concourse import mybir

    f32 = mybir.dt.float32
    f32r = mybir.dt.float32r
    A = mybir.ActivationFunctionType
    mult = mybir.AluOpType.mult
    add = mybir.AluOpType.add

    ZSPAN = 8             # lin-tiles (2 chunks each) per input DMA
    FSPAN = 2             # ogroups per output DMA

    nc = bacc.Bacc()
    zp_d = nc.dram_tensor("zp", [16, PPAD // 4], f32r, kind="ExternalInput")
    consts_d = nc.dram_tensor("consts", [128, NC_CONST], f32r, kind="ExternalInput")
    out_d = nc.dram_tensor("out", [6, PPAD], f32, kind="ExternalOutput")

    with tile.TileContext(nc) as tc:
        with tc.tile_pool(name="cpool", bufs=1) as cpool, \
             tc.tile_pool(name="zpool", bufs=3) as zpool, \
             tc.tile_pool(name="thpool", bufs=3) as thpool, \
             tc.tile_pool(name="sqpool", bufs=3) as sqpool, \
             tc.tile_pool(name="fpool", bufs=2) as fpool, \
             tc.tile_pool(name="linp", bufs=2, space="PSUM") as linp, \
             tc.tile_pool(name="pop", bufs=1, space="PSUM") as pop:

            consts = cpool.tile([128, NC_CONST], f32r)
            nc.sync.dma_start(out=consts, in_=consts_d[:, :])
            lhs2a = consts[:, 128:134]
            lhs2b = consts[:, 134:140]
            bias128 = consts[:, 142:143].bitcast(f32)
            bias_out = consts[0:6, 143:144].bitcast(f32)

            for _rep in range(repeat):
                zs = None
                F = None
                for og in range(NOG):
                    po = pop.tile([6, 4 * CHUNK], f32)
                    for lt in range(2):
                        i = og * 2 + lt
                        c0 = i * 2
                        if c0 % 16 == 0:
                            # span of 16 chunks: 4 DMAs, one per PE row group
                            span = c0 // 16
                            ncols = min(4 * CHUNK, PPAD // 4 - span * 4 * CHUNK)
                            zs = zpool.tile([128, 4 * CHUNK], f32r)
                            for g in range(4):
                                nc.sync.dma_start(
                                    out=zs[32 * g:32 * g + 4, 0:ncols],
                                    in_=zp_d[4 * g:4 * g + 4,
                                             span * 4 * CHUNK:span * 4 * CHUNK + ncols])
                        lin = linp.tile([128, 2 * CHUNK], f32)
                        for h in range(2):
                            c = c0 + h
                            g = c % 4
                            ws = (c // 4) % 4
                            nc.tensor.matmul(
                                lin[:, h * CHUNK:(h + 1) * CHUNK],
                                consts[32 * g:32 * g + 4, 0:128],
                                zs[32 * g:32 * g + 4, ws * CHUNK:(ws + 1) * CHUNK],
                                start=True, stop=True,
                                tile_position=(32 * g, 0))
                        th = thpool.tile([128, 2 * CHUNK], f32r)
                        nc.scalar.activation(th, lin, A.Tanh, bias=bias128)
                        sq = sqpool.tile([128, 2 * CHUNK], f32r)
                        if i % 2 == 0:
                            nc.vector.tensor_mul(sq, th, th)
                        else:
                            nc.gpsimd.tensor_tensor(sq, th, th, mult)
                        for c2 in range(2):
                            c = lt * 2 + c2
                            s = slice(c2 * CHUNK, (c2 + 1) * CHUNK)
                            d = slice(c * CHUNK, (c + 1) * CHUNK)
                            nc.tensor.matmul(po[:, d], lhs2a, th[:, s],
                                             start=True, stop=False)
                            nc.tensor.matmul(po[:, d], lhs2b, sq[:, s],
                                             start=False, stop=True)
                    if og % FSPAN == 0:
                        F = fpool.tile([6, FSPAN * 4 * CHUNK], f32)
                    foff = (og % FSPAN) * 4 * CHUNK
                    fv = F[:, foff:foff + 4 * CHUNK]
                    if og % 3 == 0:
                        nc.scalar.activation(fv, po, A.Identity, bias=bias_out,
                                             scale=1.0 / WIDTH)
                    else:
                        nc.vector.tensor_scalar(out=fv, in0=po,
                                                scalar1=1.0 / WIDTH,
                                                scalar2=bias_out,
                                                op0=mult, op1=add)
                    if og % FSPAN == FSPAN - 1 or og == NOG - 1:
                        og0 = og - (og % FSPAN)
                        ncols = (og - og0 + 1) * 4 * CHUNK
                        nc.sync.dma_start(
                            out=out_d[:, og0 * 4 * CHUNK:og0 * 4 * CHUNK + ncols],
                            in_=F[:, 0:ncols])
    nc.finalize()
    return nc


def _get_module():
    if "nc" not in _built:
        _built["nc"] = _build_module()
    return _built["nc"]


def _pack(np_inputs):
    """Host-side packing: hypernet + consts tile [128,144] + zp [8,4,PPAD]."""
    f = np.float32
    t = np.asarray(np_inputs["t"], f)
    z = np.asarray(np_inputs["z"], f)

    # ---- hypernetwork on host (float32, mirrors the reference) ----
    mo = np.tanh(t @ np.asarray(np_inputs["W1"], f) + np.asarray(np_inputs["b1"], f))
    mo = np.tanh(mo @ np.asarray(np_inputs["W2"], f) + np.asarray(np_inputs["b2"], f))
    u = (mo @ np.asarray(np_inputs["Wu"], f) + np.asarray(np_inputs["bu"], f)).reshape(WIDTH, D)
    w = (mo @ np.asarray(np_inputs["Ww"], f) + np.asarray(np_inputs["bw"], f)).reshape(WIDTH, D)
    bv = mo @ np.asarray(np_inputs["Wb"], f) + np.asarray(np_inputs["bb"], f)
    wu = np.sum(w * u, axis=1)          # [WIDTH]
    swu = f(wu.sum())

    # ---- consts tile [128, 144] ----
    c = np.zeros((128, NC_CONST), f)
    for g in range(4):
        c[32 * g + 0, 0:64] = w[:, 0]
        c[32 * g + 1, 0:64] = w[:, 1]
        c[32 * g + 2, 64:128] = w[:, 0]
        c[32 * g + 3, 64:128] = w[:, 1]
    # lhs2a (th contraction): cols 128..133
    c[0:64, 128] = u[:, 0]
    c[0:64, 129] = u[:, 1]
    c[64:128, 130] = u[:, 0]
    c[64:128, 131] = u[:, 1]
    # lhs2b (th^2 contraction): cols 134..139 (rows 4/5 = wu for A/B halves)
    c[0:64, 138] = wu
    c[64:128, 139] = wu
    # bias128 (col 142): +b_k per partition
    c[0:64, 142] = bv
    c[64:128, 142] = bv
    # bias_out (col 143): -sum(wu)/WIDTH on the trace rows (F rows 4, 5)
    c[4, 143] = -swu / WIDTH
    c[5, 143] = -swu / WIDTH

    # ---- pack z: rows (z0_A, z1_A, z0_B, z1_B), then regroup chunks by
    # PE row group (chunk c -> group c%4) for row-tiled MM1s ----
    zr = z.reshape(NCORES, BC, D)
    zp = np.zeros((NCORES, 4, PPAD), f)
    zp[:, 0, :PH] = zr[:, :PH, 0]
    zp[:, 1, :PH] = zr[:, :PH, 1]
    zp[:, 2, :PH] = zr[:, PH:, 0]
    zp[:, 3, :PH] = zr[:, PH:, 1]
    zp4 = zp.reshape(NCORES, 4, NCHUNK // 4, 4, CHUNK)
    zp16 = np.ascontiguousarray(zp4.transpose(0, 3, 1, 2, 4)).reshape(
        NCORES, 16, PPAD // 4)
    return zp16, c


def kernel(t, z, dlogp, W1, b1, W2, b2, Wu, bu, Ww, bw, Wb, bb):
    from concourse.bass_utils import run_bass_kernel_spmd

    f = np.float32
    zp, c = _pack(dict(t=t, z=z, W1=W1, b1=b1, W2=W2, b2=b2,
                       Wu=Wu, bu=bu, Ww=Ww, bw=bw, Wb=Wb, bb=bb))

    nc = _get_module()
    in_maps = [{"zp": np.ascontiguousarray(zp[ci]), "consts": c}
               for ci in range(NCORES)]
    res = run_bass_kernel_spmd(nc, in_maps, list(range(NCORES)))

    # ---- unpack ----
    dz = np.empty((B, D), f)
    dlp = np.empty((B, 1), f)
    dz_r = dz.reshape(NCORES, BC, D)
    dlp_r = dlp.reshape(NCORES, BC)
    for ci in range(NCORES):
        o = res.results[ci]["out"]               # [6, PPAD]
        v = o[:, :PH]
        dz_r[ci, :PH, 0] = v[0]
        dz_r[ci, :PH, 1] = v[1]
        dz_r[ci, PH:, 0] = v[2]
        dz_r[ci, PH:, 1] = v[3]
        dlp_r[ci, :PH] = v[4]
        dlp_r[ci, PH:] = v[5]
    return dz, dlp
